# revision 23
# baseline (speedup 1.0000x reference)
"""DeepseekV2 MLA attention on 8 Trainium2 NeuronCores.

Sharding: token-split A-projections (kv latents first -> early AllGather,
q latents second -> second AllGather) -> head-split (4 heads/core)
B-projections + causal attention -> per-query-block AllGather(attn out)
-> D-column-split output projection.

v2 layout notes vs baseline:
- All weights host-prepacked into the exact [128, k, cols] lhsT layout so
  every weight DMA is a contiguous stream.
- ag1 is split: kv latents (576 rows) gather while the q-chunk matmuls
  still run; q latents (1536 rows) gather while phase_b runs.
- qT stays resident in SBUF (no DRAM spill/reload); q rope is applied
  with 6 full-width DVE ops per (pair, qblock) writing qpe_sb directly.
- Attention denominators accumulate in bf16 on DVE (one ones-matmul per
  (qb, head) instead of one per (kk, head)); broadcast matmuls run bf16.
- Attention runs qb=3..0 so the last ag2 fires early and the output
  projection tail hides under remaining PE work.

Precision: bf16 matmul inputs with fp32 PSUM accumulation; rmsnorm
statistics in fp32/f32r; softmax denominators accumulate bf16 partials
into fp32 PSUM via a ones-matmul.
"""
import math

import numpy as np
import ml_dtypes

import concourse.bass as bass
import concourse.mybir as mybir
from concourse.tile import TileContext
from concourse import bass_utils

# ---------------------------------------------------------------------------
# Walrus workaround: this container's walrus accepts at most ONE sync-wait
# per TPB instruction, but Tile attaches several (tail Drain, LDWEIGHTS...).
# Split: keep the last wait, move the rest onto preceding same-engine NOPs.
# ---------------------------------------------------------------------------
import concourse.tile as _tile_mod

_orig_sched = _tile_mod.TileContext.schedule_and_allocate
_nopctr = [0]


def _split_multiwait(nc):
    for fn in nc.m.functions:
        for blk in fn.blocks:
            insts = blk.instructions
            if not any(
                i.sync_info and i.sync_info.on_wait and len(i.sync_info.on_wait) > 1
                for i in insts
            ):
                continue
            out = []
            for ins in insts:
                si = ins.sync_info
                if si and si.on_wait and len(si.on_wait) > 1:
                    waits = list(si.on_wait)
                    for w in waits[:-1]:
                        _nopctr[0] += 1
                        nop = mybir.InstNoOp(name=f"I-mws-{_nopctr[0]}", ins=[], outs=[])
                        nop.engine = ins.engine
                        nop.sync_info = mybir.SyncInfo(on_wait=[w], on_update=[])
                        out.append(nop)
                    ins.sync_info = mybir.SyncInfo(
                        on_wait=[waits[-1]], on_update=list(si.on_update or [])
                    )
                out.append(ins)
            blk.instructions = out


def _patched_sched(self, *a, **k):
    res = _orig_sched(self, *a, **k)
    _split_multiwait(self.nc)
    return res


if getattr(_tile_mod.TileContext.schedule_and_allocate, "__name__", "") != "_patched_sched":
    _tile_mod.TileContext.schedule_and_allocate = _patched_sched


# ---------------------------------------------------------------------------
T, D, H = 2048, 5120, 32
NOPE, ROPE, QK = 128, 64, 192
KVR, QR, VH = 512, 1536, 128
EPS, THETA = 1e-6, 10000.0
NCORES = 8
HL = H // NCORES          # 4 heads per core
TC = T // NCORES          # 256 tokens per core
LAT = KVR + ROPE          # 576 rows in the kv allgather
DCOL = D // NCORES        # 640 output columns per core
NKC = D // 128            # 40 contraction chunks for the A matmuls

F32 = mybir.dt.float32
F32R = mybir.dt.float32r
BF16 = mybir.dt.bfloat16
AF = mybir.ActivationFunctionType
MUL = mybir.AluOpType.mult
ADD = mybir.AluOpType.add
SUB = mybir.AluOpType.subtract

TRACE = [False]          # test.py sets TRACE[0]=True to profile
LAST_RESULT = [None]     # BassKernelResults stashed here for test.py

_cache = {}

# phase_a chunk table: (kind, mrows); chunk c covers wa cols
# [off(c), off(c)+40*mrows).  kv chunks first so ag1b can fire early.
_ACH = [("kv", 128)] * 4 + [("pe", 64)] + [("q", 128)] * 12
_AOFF = np.concatenate([[0], np.cumsum([NKC * m for _, m in _ACH])]).tolist()


def _phase_a_kv(nc, tc, io, consts_t, pools, ag1b_in):
    """Token-split kv-latent A projection, rmsnorm, k_pe rope -> ag1b_in."""
    a_ht, a_w, a_st, a_tmp, a_ps, a_ss = pools
    ones_cb, ones_r = consts_t["ones_cb"], consts_t["ones_r"]
    bias_sb = consts_t["bias_sb"]

    ht_sb = a_ht.tile([128, NKC * TC], BF16, name="ht_sb")
    htv = ht_sb[:].rearrange("p (k t) -> p k t", k=NKC)
    htp = io["hTp"][:].rearrange("p (k t) -> p k t", k=NKC)
    # DMA engines run well below peak while the device warms up, so the
    # first weight chunk and first ht chunk are finely interleaved: the
    # first matmuls become runnable after ~0.6MB instead of ~1.9MB.
    wt0 = a_w.tile([128, NKC * 128], BF16, name="a_w_0", tag="aw")
    nc.sync.dma_start(wt0[:, 0:5 * 128], io["wa"][:, 0:5 * 128])
    nc.sync.dma_start(htv[:, 0:4, :], htp[:, 0:4, :])
    nc.sync.dma_start(wt0[:, 5 * 128:10 * 128], io["wa"][:, 5 * 128:10 * 128])
    nc.sync.dma_start(htv[:, 4:8, :], htp[:, 4:8, :])
    for part in range(1, 4):
        lo, hi = part * (NKC // 4) * 128, (part + 1) * (NKC // 4) * 128
        nc.sync.dma_start(wt0[:, lo:hi], io["wa"][:, lo:hi])
    for g in range(1, 5):
        nc.sync.dma_start(htv[:, 8 * g:8 * (g + 1), :], htp[:, 8 * g:8 * (g + 1), :])
    stage = a_st.tile([128, 5 * TC], F32R, name="stage")
    ss_kv = a_ss.tile([1, TC], F32, name="ss_kv")
    consts_t["htv"] = htv
    consts_t["stage"] = stage

    for c in range(5):
        kind, mrows = _ACH[c]
        if c == 0:
            wt = wt0
        else:
            wt = a_w.tile([128, NKC * 128], BF16, name=f"a_w_{c}", tag="aw")
            nc.sync.dma_start(wt[:, :NKC * mrows],
                              io["wa"][:, _AOFF[c]:_AOFF[c + 1]])
        wtv = wt[:, :NKC * mrows].rearrange("p (k c) -> p k c", k=NKC)
        ps = a_ps.tile([128, TC], F32, name=f"a_ps_{c}", tag="aps")
        for k in range(NKC):
            nc.tensor.matmul(ps[:mrows, :], wtv[:, k, :], htv[:, k, :],
                             start=(k == 0), stop=(k == NKC - 1))
        if c == 0:
            # rope tables for the k_pe rope below; emitted here so they
            # trail the first weight chunk in the DMA queue
            for nm, srcn in (("cosa_sb", "cosA"), ("sina_sb", "sinA")):
                nc.sync.dma_start(consts_t[nm][:], io[srcn][:])
        st = stage[:, c * TC:(c + 1) * TC]
        if kind == "kv":
            nc.vector.tensor_scalar(st, ps[:], bias_sb[:, c:c + 1], None, op0=ADD)
            sq = a_tmp.tile([128, TC], BF16, name=f"sq_{c}", tag="sq")
            nc.scalar.activation(sq[:], st, AF.Square)
            nc.tensor.matmul(ss_kv[:], ones_cb, sq[:],
                             start=(c == 0), stop=(c == 3))
        else:
            nc.vector.tensor_scalar(st[:64, :], ps[:64, :],
                                    bias_sb[:64, 4:5], None, op0=ADD)

    # kv rms scale: 1/sqrt(mean(ss) + eps) broadcast to 128 partitions
    bc_kv = _rms_scale(nc, consts_t, a_tmp, a_ps, ss_kv, KVR, "kv")
    for c in range(4):
        st = stage[:, c * TC:(c + 1) * TC]
        sc = a_tmp.tile([128, TC], BF16, name=f"sc_kv_{c}", tag="sc")
        nc.vector.tensor_tensor(sc[:], st, bc_kv[:], op=MUL)
        nc.sync.dma_start(ag1b_in[c * 128:(c + 1) * 128, :], sc[:])

    # k_pe rope (no norm) -> rows 512:576
    cosa_sb, sina_sb = consts_t["cosa_sb"], consts_t["sina_sb"]
    st = stage[:, 4 * TC:5 * TC]
    rp = a_tmp.tile([64, TC], BF16, name="rp_kpe")
    t1 = a_tmp.tile([32, TC], F32R, name="rt1", tag="rt1")
    t2 = a_tmp.tile([32, TC], F32R, name="rt2", tag="rt2")
    x1, x2 = st[0:32, :], st[32:64, :]
    nc.vector.tensor_tensor(t1[:], x1, cosa_sb[0:32, :], op=MUL)
    nc.vector.tensor_tensor(t2[:], x2, sina_sb[32:64, :], op=MUL)
    nc.vector.tensor_tensor(rp[0:32, :], t1[:], t2[:], op=SUB)
    nc.vector.tensor_tensor(t1[:], x1, sina_sb[0:32, :], op=MUL)
    nc.vector.tensor_tensor(t2[:], x2, cosa_sb[32:64, :], op=MUL)
    nc.vector.tensor_tensor(rp[32:64, :], t1[:], t2[:], op=ADD)
    nc.sync.dma_start(ag1b_in[512:576, :], rp[:])


def _phase_a_q(nc, tc, io, consts_t, pools, ag1a1_in, ag1a2_in, fire1):
    """Token-split q-latent A projection, staged UNNORMALIZED.

    The per-token rms scale rs ships as row 768 of ag1a2; phase_q applies
    it after its matmuls (the norm is linear in the latent).  fire1() is
    called once the first six chunks are staged, so the first half
    gathers while the rest computes.
    """
    a_ht, a_w, a_st, a_tmp, a_ps, a_ss = pools
    ones_cb = consts_t["ones_cb"]
    htv = consts_t["htv"]
    ss_q = a_ss.tile([1, TC], F32, name="ss_q")

    for c in range(5, 17):
        wt = a_w.tile([128, NKC * 128], BF16, name=f"a_w_{c}", tag="aw")
        nc.sync.dma_start(wt[:], io["wa"][:, _AOFF[c]:_AOFF[c + 1]])
        wtv = wt[:].rearrange("p (k c) -> p k c", k=NKC)
        ps = a_ps.tile([128, TC], F32, name=f"a_ps_{c}", tag="aps")
        for k in range(NKC):
            nc.tensor.matmul(ps[:], wtv[:, k, :], htv[:, k, :],
                             start=(k == 0), stop=(k == NKC - 1))
        sq = a_tmp.tile([128, TC], BF16, name=f"sq_{c}", tag="sq")
        nc.scalar.activation(sq[:], ps[:], AF.Square)
        nc.tensor.matmul(ss_q[:], ones_cb, sq[:],
                         start=(c == 5), stop=(c == 16))
        sc = a_tmp.tile([128, TC], BF16, name=f"sc_q_{c}", tag="sc")
        nc.vector.tensor_copy(sc[:], ps[:])
        if c < 11:
            nc.sync.dma_start(ag1a1_in[(c - 5) * 128:(c - 4) * 128, :], sc[:])
            if c == 10:
                fire1()
        else:
            nc.sync.dma_start(ag1a2_in[(c - 11) * 128:(c - 10) * 128, :], sc[:])

    # rs = 1/sqrt(mean(ss)+eps), shipped bf16 as row 768 of ag1a2
    ms = a_tmp.tile([1, TC], F32R, name="ms_q", tag="ms")
    nc.vector.tensor_scalar(ms[:], ss_q[:], 1.0 / QR, EPS, op0=MUL, op1=ADD)
    sq2 = a_tmp.tile([1, TC], F32R, name="sqr_q", tag="sqr")
    nc.scalar.activation(sq2[:], ms[:], AF.Sqrt)
    rs = a_tmp.tile([1, TC], F32R, name="rs_q", tag="rs")
    with nc.allow_low_precision(reason="f32r holds full fp32 bits"):
        nc.vector.reciprocal(rs[:], sq2[:])
    rsb = a_tmp.tile([1, TC], BF16, name="rsb_q", tag="rsb")
    nc.vector.tensor_copy(rsb[:], rs[:])
    nc.sync.dma_start(ag1a2_in[768:769, :], rsb[:])


def _rms_scale(nc, consts_t, a_tmp, a_ps, ss, nfeat, key):
    """1/sqrt(mean(ss)+eps) broadcast to [128, TC] f32r."""
    ms = a_tmp.tile([1, TC], F32R, name=f"ms_{key}", tag="ms")
    nc.vector.tensor_scalar(ms[:], ss[:], 1.0 / nfeat, EPS, op0=MUL, op1=ADD)
    sq2 = a_tmp.tile([1, TC], F32R, name=f"sqr_{key}", tag="sqr")
    nc.scalar.activation(sq2[:], ms[:], AF.Sqrt)
    rs = a_tmp.tile([1, TC], F32R, name=f"rs_{key}", tag="rs")
    with nc.allow_low_precision(reason="f32r holds full fp32 bits"):
        nc.vector.reciprocal(rs[:], sq2[:])
    bps = a_ps.tile([128, TC], F32, name=f"bps_{key}", tag="bps")
    nc.tensor.matmul(bps[:], consts_t["ones_r"][:1, :], rs[:],
                     start=True, stop=True)
    bc = a_tmp.tile([128, TC], F32R, name=f"bc_{key}", tag=f"bc{key}")
    nc.vector.tensor_copy(bc[:], bps[:])
    return bc


def _phase_b(nc, tc, io, ag1bv, ktv, vv, kpe_sb):
    """Head-split k_nope^T and v projections from the gathered kv latents."""
    with (
        tc.tile_pool(name="b_kva", bufs=1) as b_kva,
        tc.tile_pool(name="b_w", bufs=2) as b_w,
        tc.tile_pool(name="b_ps", bufs=2, space="PSUM") as b_ps,
    ):
        kva_sb = b_kva.tile([128, 4 * T], BF16, name="kva_sb")
        kvav = kva_sb[:].rearrange("p (k t) -> p k t", k=4)
        for k in range(4):
            nc.gpsimd.dma_start(
                kvav[:, k, :].rearrange("p (r t) -> p r t", r=NCORES),
                ag1bv[k * 128:(k + 1) * 128])
        nc.gpsimd.dma_start(
            kpe_sb[:].rearrange("p (r t) -> p r t", r=NCORES),
            ag1bv[512:576])

        wk_sb = b_w.tile([128, 4 * 512], BF16, name="wk_sb", tag="wkw")
        wkv_ = wk_sb[:].rearrange("p (k c) -> p k c", k=4)
        nc.sync.dma_start(wk_sb[:], io["wkvbk"][:])
        for j in range(HL):
            for qb in range(4):
                ps = b_ps.tile([128, 512], F32, name=f"psk_{j}_{qb}", tag="psk")
                for k in range(4):
                    nc.tensor.matmul(ps[:], wkv_[:, k, j * 128:(j + 1) * 128],
                                     kvav[:, k, qb * 512:(qb + 1) * 512],
                                     start=(k == 0), stop=(k == 3))
                nc.vector.tensor_copy(ktv[:, j, qb * 512:(qb + 1) * 512], ps[:])

        wv_sb = b_w.tile([128, 4 * 512], BF16, name="wv_sb", tag="wvw")
        wvv = wv_sb[:].rearrange("p (k c) -> p k c", k=4)
        nc.sync.dma_start(wv_sb[:], io["wkvbv"][:])
        for mt in range(16):
            ps = b_ps.tile([128, 512], F32, name=f"psv_{mt}", tag="psv")
            for k in range(4):
                nc.tensor.matmul(ps[:], kvav[:, k, mt * 128:(mt + 1) * 128],
                                 wvv[:, k, :], start=(k == 0), stop=(k == 3))
            nc.vector.tensor_copy(vv[:, mt, :], ps[:])


def _phase_q(nc, tc, io, consts_t, ag1a1v, ag1a2v, qtv, qpev):
    """Head-split q^T projection from unnormalized latents; rope pairs
    first; the per-token rms scale rs is broadcast and applied here."""
    cos_sb, sin_sb = consts_t["cos_sb"], consts_t["sin_sb"]
    ones_rb = consts_t["ones_rb"]
    with (
        tc.tile_pool(name="c_qa", bufs=1) as c_qa,
        tc.tile_pool(name="c_w", bufs=2) as c_w,
        tc.tile_pool(name="c_tmp", bufs=3) as c_tmp,
        tc.tile_pool(name="c_rs", bufs=1) as c_rs,
        tc.tile_pool(name="c_ps", bufs=3, space="PSUM") as c_ps,
        tc.tile_pool(name="c_bps", bufs=1, space="PSUM") as c_bps,
    ):
        qa_sb = c_qa.tile([128, 12 * T], BF16, name="qa_sb")
        qav = qa_sb[:].rearrange("p (k t) -> p k t", k=12)
        for k in range(12):
            srcv = (ag1a1v[k * 128:(k + 1) * 128] if k < 6 else
                    ag1a2v[(k - 6) * 128:(k - 5) * 128])
            nc.gpsimd.dma_start(
                qav[:, k, :].rearrange("p (r t) -> p r t", r=NCORES), srcv)
        rs_sb = c_rs.tile([1, T], BF16, name="rs_sb")
        nc.gpsimd.dma_start(
            rs_sb[:].rearrange("p (r t) -> p r t", r=NCORES),
            ag1a2v[768:769])
        rsb = c_rs.tile([128, T], BF16, name="rsb")

        for m in range(6):
            wt = c_w.tile([128, 12 * 128], BF16, name=f"cw_{m}", tag="cw")
            nc.sync.dma_start(wt[:], io["wqb"][:, m * 1536:(m + 1) * 1536])
            wtv = wt[:].rearrange("p (k c) -> p k c", k=12)
            for pair in range(2):
                pss = [c_ps.tile([128, 512], F32, name=f"psq_{m}_{qb}",
                                 tag=f"psq{qb % 2}")
                       for qb in (2 * pair, 2 * pair + 1)]
                for k in range(12):
                    for s in range(2):
                        qb = 2 * pair + s
                        nc.tensor.matmul(pss[s][:], wtv[:, k, :],
                                         qav[:, k, qb * 512:(qb + 1) * 512],
                                         start=(k == 0), stop=(k == 11))
                if m == 0 and pair == 0:
                    # rs broadcast to 128 partitions, once; then folded
                    # straight into the rope tables so each rope block
                    # saves two full-width DVE muls
                    for b in range(4):
                        bp = c_bps.tile([128, 512], F32, name=f"rsbp_{b}",
                                        tag="rsbp")
                        nc.tensor.matmul(bp[:], ones_rb[:1, :],
                                         rs_sb[:, b * 512:(b + 1) * 512],
                                         start=True, stop=True)
                        nc.vector.tensor_copy(
                            rsb[:, b * 512:(b + 1) * 512], bp[:])
                    nc.vector.tensor_tensor(cos_sb[:], cos_sb[:], rsb[:],
                                            op=MUL)
                    nc.vector.tensor_tensor(sin_sb[:], sin_sb[:], rsb[:],
                                            op=MUL)
                for s in range(2):
                    qb = 2 * pair + s
                    cols = slice(qb * 512, (qb + 1) * 512)
                    if m < 2:
                        # rope pair m: heads (2m, 2m+1); pss rows are
                        # [x1h0, x1h1, x2h0, x2h1].  A,B carry the rs
                        # scale; Bs halves are swapped so every combine
                        # reads both inputs from the same base partition.
                        A = c_tmp.tile([128, 512], F32R, name=f"ra_{m}_{qb}",
                                       tag="ra")
                        B = c_tmp.tile([128, 512], F32R, name=f"rb_{m}_{qb}",
                                       tag="rb")
                        nc.vector.tensor_tensor(A[:], pss[s][:],
                                                cos_sb[:, cols], op=MUL)
                        nc.vector.tensor_tensor(B[0:64, :], pss[s][64:128, :],
                                                sin_sb[64:128, cols], op=MUL)
                        nc.vector.tensor_tensor(B[64:128, :], pss[s][0:64, :],
                                                sin_sb[0:64, cols], op=MUL)
                        for hh in range(2):
                            j = 2 * m + hh
                            r0 = 32 * hh
                            nc.vector.tensor_tensor(
                                qpev[0:32, j, cols], A[r0:r0 + 32, :],
                                B[r0:r0 + 32, :], op=SUB)
                            nc.vector.tensor_tensor(
                                qpev[32:64, j, cols], A[64 + r0:64 + r0 + 32, :],
                                B[64 + r0:64 + r0 + 32, :], op=ADD)
                    else:
                        nc.vector.tensor_tensor(qtv[:, m - 2, cols],
                                                pss[s][:], rsb[:, cols],
                                                op=MUL)


def _phase_attn_out(nc, tc, io, qtv, qpev, ag2_ins, ag2_outs, ktv, vv,
                    kpe_sb, wov, consts_t):
    """Causal attention (qb descending, software-pipelined ots) with the
    output-projection token blocks interleaved into the PE issue order:
    qb3, qb2, OUT3, qb1, OUT2, qb0, OUT1, OUT0."""
    ones_cb, ones_rb, tri_sb = (consts_t["ones_cb"], consts_t["ones_rb"],
                                consts_t["tri_b"])
    with (
        tc.tile_pool(name="t_p", bufs=6) as t_p,
        tc.tile_pool(name="t_den", bufs=2) as t_den,
        tc.tile_pool(name="t_o", bufs=2) as t_o,
        tc.tile_pool(name="t_ps", bufs=3, space="PSUM") as t_ps,
        tc.tile_pool(name="t_db", bufs=1, space="PSUM") as t_db,
        tc.tile_pool(name="t_acc", bufs=1, space="PSUM") as t_acc,
        tc.tile_pool(name="o_a", bufs=3) as o_a,
        tc.tile_pool(name="o_st", bufs=2) as o_st,
        tc.tile_pool(name="o_ps", bufs=2, space="PSUM") as o_ps,
    ):
        def attn_qb(qb, fillers=()):
            fillers = list(fillers)
            for jp in range(HL // 2):
                js = (2 * jp, 2 * jp + 1)
                dacs, ots, prev = {}, {}, {}
                for s, j in enumerate(js):
                    dacs[j] = t_den.tile([128, 512], BF16, name=f"dac_{qb}_{j}",
                                         tag=f"dac{s}")
                    ots[j] = t_acc.tile([128, 512], F32, name=f"ot_{qb}_{j}",
                                        tag=f"ot{s}")
                kmax = 4 * qb + 4
                for kk in range(kmax):
                    o = kk - 4 * qb
                    c0 = max(0, o) * 128
                    pts = {}
                    for s, j in enumerate(js):
                        sT = t_ps.tile([128, 512], F32,
                                       name=f"sT_{qb}_{j}_{kk}", tag="sT")
                        nc.tensor.matmul(sT[:, c0:512],
                                         ktv[:, j, kk * 128:(kk + 1) * 128],
                                         qtv[:, j, qb * 512 + c0:(qb + 1) * 512],
                                         start=True, stop=False)
                        nc.tensor.matmul(sT[:, c0:512],
                                         kpe_sb[:, kk * 128:(kk + 1) * 128],
                                         qpev[:, j, qb * 512 + c0:(qb + 1) * 512],
                                         start=False, stop=True)
                        pT = t_p.tile([128, 512], BF16,
                                      name=f"pT_{qb}_{j}_{kk}", tag="pT")
                        nc.scalar.activation(pT[:, c0:512], sT[:, c0:512],
                                             AF.Exp)
                        if o >= 0:
                            nc.vector.tensor_tensor(pT[:, c0:c0 + 128],
                                                    pT[:, c0:c0 + 128],
                                                    tri_sb[:], op=MUL)
                        pts[j] = pT
                    # deferred PV matmuls from the previous kk so the exp
                    # (Scalar) of this kk hides under PE work
                    if kk > 0:
                        for j in js:
                            pc0, pT0 = prev[j]
                            nc.tensor.matmul(ots[j][:, pc0:512],
                                             vv[:, kk - 1, j * 128:(j + 1) * 128],
                                             pT0[:, pc0:512],
                                             start=(kk == 1), stop=False)
                    for j in js:
                        pT = pts[j]
                        if kk == 0:
                            nc.vector.tensor_copy(dacs[j][:], pT[:])
                        else:
                            nc.vector.tensor_tensor(dacs[j][:, c0:512],
                                                    dacs[j][:, c0:512],
                                                    pT[:, c0:512], op=ADD)
                        prev[j] = (c0, pT)
                for j in js:
                    pc0, pT0 = prev[j]
                    nc.tensor.matmul(ots[j][:, pc0:512],
                                     vv[:, kmax - 1, j * 128:(j + 1) * 128],
                                     pT0[:, pc0:512],
                                     start=(kmax == 1), stop=True)
                if fillers:
                    fillers.pop(0)()
                for s, j in enumerate(js):
                    denp = t_db.tile([1, 512], F32, name=f"dp_{qb}_{j}",
                                     tag="db")
                    nc.tensor.matmul(denp[:], ones_cb, dacs[j][:],
                                     start=True, stop=True)
                    rden = t_o.tile([1, 512], F32R, name=f"rden_{qb}_{j}",
                                    tag=f"rden{s}")
                    with nc.allow_low_precision(reason="f32r = fp32 bits"):
                        nc.vector.reciprocal(rden[:], denp[:])
                    rdb = t_o.tile([1, 512], BF16, name=f"rdb_{qb}_{j}",
                                   tag=f"rdb{s}")
                    nc.vector.tensor_copy(rdb[:], rden[:])
                    bcp = t_db.tile([128, 512], F32, name=f"bcp_{qb}_{j}",
                                    tag="db")
                    nc.tensor.matmul(bcp[:], ones_rb[:1, :], rdb[:],
                                     start=True, stop=True)
                    bcs = t_o.tile([128, 512], F32R, name=f"bcs_{qb}_{j}",
                                   tag=f"bcs{s}")
                    nc.vector.tensor_copy(bcs[:], bcp[:])
                    obf = t_o.tile([128, 512], BF16, name=f"obf_{qb}_{j}",
                                   tag=f"obf{s}")
                    nc.vector.tensor_tensor(obf[:], ots[j][:], bcs[:], op=MUL)
                    nc.sync.dma_start(
                        ag2_ins[qb][j * 128:(j + 1) * 128, :], obf[:])
            nc.gpsimd.collective_compute(
                "AllGather", mybir.AluOpType.bypass,
                ins=[ag2_ins[qb][:]], outs=[ag2_outs[qb][:]],
                replica_groups=[list(range(NCORES))],
            )

        def out_prep(tq):
            oavs = []
            for h in range(2):
                oa = o_a.tile([128, 16 * 512], BF16, name=f"oa_{tq}_{h}",
                              tag="oa")
                oav = oa[:].rearrange("p (k t) -> p k t", k=16)
                nc.gpsimd.dma_start(
                    oav, ag2_outs[tq][:].rearrange("(k p) t -> p k t", p=128)
                    [:, 16 * h:16 * (h + 1), :])
                oavs.append(oav)
            return oavs

        def out_d(tq, oavs, d):
            ps = o_ps.tile([128, 512], F32, name=f"ops_{tq}_{d}", tag="ops")
            for k in range(32):
                nc.tensor.matmul(ps[:], wov[:, k, d * 128:(d + 1) * 128],
                                 oavs[k // 16][:, k % 16, :],
                                 start=(k == 0), stop=(k == 31))
            st = o_st.tile([128, 512], F32, name=f"ost_{tq}_{d}", tag="ost")
            nc.vector.tensor_copy(st[:], ps[:])
            nc.sync.dma_start(
                io["outT"][d * 128:(d + 1) * 128,
                           tq * 512:(tq + 1) * 512], st[:])

        attn_qb(3)
        attn_qb(2)
        p3 = out_prep(3)
        attn_qb(1, [lambda: out_d(3, p3, 0), lambda: out_d(3, p3, 1)])
        p2 = out_prep(2)
        attn_qb(0, [lambda: out_d(3, p3, 2), lambda: out_d(3, p3, 3)])
        out_d(3, p3, 4)
        for d in range(5):
            out_d(2, p2, d)
        p1 = out_prep(1)
        for d in range(5):
            out_d(1, p1, d)
        p0 = out_prep(0)
        for d in range(5):
            out_d(0, p0, d)


def _build():
    nc = bass.Bass("TRN2", target_bir_lowering=False, debug=False,
                   num_devices=NCORES)
    io = {
        "hTp": nc.dram_tensor("hTp", [128, NKC * TC], BF16, kind="ExternalInput"),
        "wa": nc.dram_tensor("wa", [128, _AOFF[-1]], BF16, kind="ExternalInput"),
        "biask": nc.dram_tensor("biask", [128, 5], F32, kind="ExternalInput"),
        "wqb": nc.dram_tensor("wqb", [128, 6 * 1536], BF16, kind="ExternalInput"),
        "wkvbk": nc.dram_tensor("wkvbk", [128, 4 * 512], BF16,
                                kind="ExternalInput"),
        "wkvbv": nc.dram_tensor("wkvbv", [128, 4 * 512], BF16,
                                kind="ExternalInput"),
        "wo": nc.dram_tensor("wo", [128, 32 * DCOL], BF16, kind="ExternalInput"),
        "cosTb": nc.dram_tensor("cosTb", [128, T], BF16, kind="ExternalInput"),
        "sinTb": nc.dram_tensor("sinTb", [128, T], BF16, kind="ExternalInput"),
        "cosA": nc.dram_tensor("cosA", [128, TC], F32R, kind="ExternalInput"),
        "sinA": nc.dram_tensor("sinA", [128, TC], F32R, kind="ExternalInput"),
        "tri": nc.dram_tensor("tri", [128, 128], F32R, kind="ExternalInput"),
        "onesin": nc.dram_tensor("onesin", [128, 128], F32R, kind="ExternalInput"),
        "outT": nc.dram_tensor("outT", [DCOL, T], F32, kind="ExternalOutput"),
    }

    with TileContext(nc) as tc:
        with (
            tc.tile_pool(name="dram", bufs=1, space="DRAM") as dram,
            tc.tile_pool(name="consts", bufs=1) as consts,
            tc.tile_pool(name="persist", bufs=1) as persist,
        ):
            dum_in = dram.tile([1, 16], BF16, name="dum_in")
            dum_out = dram.tile([NCORES, 16], BF16, addr_space="Shared",
                                name="dum_out")
            ag1a1_in = dram.tile([768, TC], BF16, name="ag1a1_in")
            ag1a1_out = dram.tile([NCORES * 768, TC], BF16, addr_space="Shared",
                                  name="ag1a1_out")
            ag1a2_in = dram.tile([769, TC], BF16, name="ag1a2_in")
            ag1a2_out = dram.tile([NCORES * 769, TC], BF16, addr_space="Shared",
                                  name="ag1a2_out")
            ag1b_in = dram.tile([LAT, TC], BF16, name="ag1b_in")
            ag1b_out = dram.tile([NCORES * LAT, TC], BF16, addr_space="Shared",
                                 name="ag1b_out")
            ag2_ins = [dram.tile([HL * VH, 512], BF16, name=f"ag2_in_{qb}")
                       for qb in range(4)]
            ag2_outs = [dram.tile([H * VH, 512], BF16, addr_space="Shared",
                                  name=f"ag2_out_{qb}") for qb in range(4)]

            def fire_ag(name, tin, tout):
                with nc.named_scope(name):
                    nc.gpsimd.collective_compute(
                        "AllGather", mybir.AluOpType.bypass,
                        ins=[tin[:]], outs=[tout[:]],
                        replica_groups=[list(range(NCORES))],
                    )

            # absorb cross-core launch skew before real collectives
            zb = consts.tile([1, 16], BF16, name="zb")
            nc.vector.memset(zb[:], 0)
            nc.sync.dma_start(dum_in[:], zb[:])
            fire_ag("dummy", dum_in, dum_out)

            consts_t = {}
            ones_sb = consts.tile([128, 128], F32R, name="ones_sb")
            nc.sync.dma_start(ones_sb[:], io["onesin"][:])
            consts_t["ones_r"] = ones_sb
            ones_b = consts.tile([128, 128], BF16, name="ones_b")
            nc.vector.tensor_copy(ones_b[:], ones_sb[:])
            consts_t["ones_cb"] = ones_b[:, 0:1]
            consts_t["ones_rb"] = ones_b
            consts_t["bias_sb"] = consts.tile([128, 5], F32, name="bias_sb")
            nc.sync.dma_start(consts_t["bias_sb"][:], io["biask"][:])
            for nm, shp in (("tri_sb", [128, 128]),
                            ("cosa_sb", [128, TC]),
                            ("sina_sb", [128, TC])):
                consts_t[nm] = consts.tile(shp, F32R, name=nm)
            trib = consts.tile([128, 128], BF16, name="trib")
            consts_t["tri_b"] = trib

            # persistent SBUF tensors
            kt_sb = persist.tile([128, HL * T], BF16, name="kt_sb")
            ktv = kt_sb[:].rearrange("p (j t) -> p j t", j=HL)
            v_sb = persist.tile([128, 16 * 512], BF16, name="v_sb")
            vv = v_sb[:].rearrange("p (mt c) -> p mt c", mt=16)
            kpe_sb = persist.tile([64, T], BF16, name="kpe_sb")
            qt_sb = persist.tile([128, 4 * T], BF16, name="qt_sb")
            qtv = qt_sb[:].rearrange("p (j t) -> p j t", j=4)
            qpe_sb = persist.tile([64, HL * T], BF16, name="qpe_sb")
            qpev = qpe_sb[:].rearrange("p (j t) -> p j t", j=HL)
            wo_sb = persist.tile([128, 32 * DCOL], BF16, name="wo_sb")
            wov = wo_sb[:].rearrange("p (k c) -> p k c", k=32)

            with (
                tc.tile_pool(name="a_ht", bufs=1) as a_ht,
                tc.tile_pool(name="a_w", bufs=3) as a_w,
                tc.tile_pool(name="a_st", bufs=1) as a_st,
                tc.tile_pool(name="a_tmp", bufs=2) as a_tmp,
                tc.tile_pool(name="a_ps", bufs=2, space="PSUM") as a_ps,
                tc.tile_pool(name="a_ss", bufs=1, space="PSUM") as a_ss,
            ):
                pools = (a_ht, a_w, a_st, a_tmp, a_ps, a_ss)
                with nc.named_scope("phase_a"):
                    _phase_a_kv(nc, tc, io, consts_t, pools, ag1b_in)
                fire_ag("ag1b", ag1b_in, ag1b_out)
                with nc.named_scope("phase_a2"):
                    _phase_a_q(nc, tc, io, consts_t, pools, ag1a1_in,
                               ag1a2_in,
                               lambda: fire_ag("ag1a1", ag1a1_in, ag1a1_out))
                fire_ag("ag1a2", ag1a2_in, ag1a2_out)
            ag1a1v = ag1a1_out[:].rearrange("(r a) t -> a r t", a=768)
            ag1a2v = ag1a2_out[:].rearrange("(r a) t -> a r t", a=769)
            ag1bv = ag1b_out[:].rearrange("(r a) t -> a r t", a=LAT)

            nc.gpsimd.dma_start(wo_sb[:], io["wo"][:])
            with nc.named_scope("phase_b"):
                _phase_b(nc, tc, io, ag1bv, ktv, vv, kpe_sb)
            # late consts: rope tables for phase_q, mask for attn
            for nm, srcn in (("cos_sb", "cosTb"), ("sin_sb", "sinTb")):
                consts_t[nm] = consts.tile([128, T], BF16, name=nm)
                nc.sync.dma_start(consts_t[nm][:], io[srcn][:])
            nc.sync.dma_start(consts_t["tri_sb"][:], io["tri"][:])
            nc.vector.tensor_copy(trib[:], consts_t["tri_sb"][:])
            with nc.named_scope("phase_q"):
                _phase_q(nc, tc, io, consts_t, ag1a1v, ag1a2v, qtv, qpev)
            with nc.named_scope("phase_attn"):
                _phase_attn_out(nc, tc, io, qtv, qpev, ag2_ins, ag2_outs,
                                ktv, vv, kpe_sb, wov, consts_t)
    return nc


def _get_nc():
    if "nc" not in _cache:
        _cache["nc"] = _build()
    return _cache["nc"]


def _pack_lhsT(w, ncols=None):
    """[D, C] weight -> [128, (D/128)*C] bf16, k-major per-partition cols."""
    d, c = w.shape
    arr = w.reshape(d // 128, 128, c).transpose(1, 0, 2).reshape(128, -1)
    return np.ascontiguousarray(arr).astype(ml_dtypes.bfloat16)


def _prep(inputs):
    h = np.asarray(inputs["h"], np.float32)
    pos = np.asarray(inputs["position_ids"], np.int32)
    Wq_a = np.asarray(inputs["Wq_a"], np.float32)
    gq = np.asarray(inputs["gq"], np.float32)
    Wq_b = np.asarray(inputs["Wq_b"], np.float32)
    Wkv_a = np.asarray(inputs["Wkv_a"], np.float32)
    bkv_a = np.asarray(inputs["bkv_a"], np.float32)
    gkv = np.asarray(inputs["gkv"], np.float32)
    Wkv_b = np.asarray(inputs["Wkv_b"], np.float32)
    Wo = np.asarray(inputs["Wo"], np.float32)

    dperm = np.concatenate([np.arange(0, ROPE, 2), np.arange(1, ROPE, 2)])
    scale = np.float32(1.0 / math.sqrt(QK))

    # phase_a weights: kv chunks, k_pe chunk, q chunks (packed lhsT layout)
    wa_chunks = []
    for c in range(4):
        wa_chunks.append(_pack_lhsT(Wkv_a[:, c * 128:(c + 1) * 128]))
    wa_chunks.append(_pack_lhsT(Wkv_a[:, KVR + dperm]))
    for c in range(12):
        wa_chunks.append(_pack_lhsT(Wq_a[:, c * 128:(c + 1) * 128]))
    wa = np.concatenate(wa_chunks, axis=1)

    bias = np.zeros((128, 5), np.float32)
    bias[:, :4] = bkv_a[:KVR].reshape(4, 128).T
    bias[:64, 4] = bkv_a[KVR + dperm]

    wqb_eff = (Wq_b * gq[:, None]) * scale              # [QR, H*QK]
    wkvb_eff = Wkv_b * gkv[:, None]                     # [KVR, H*(NOPE+VH)]

    inv = THETA ** (-np.arange(0, ROPE, 2, dtype=np.float32) / ROPE)
    fr = pos.astype(np.float32)[:, None] * inv[None, :]  # [T, 32]
    cosT = np.ascontiguousarray(np.tile(np.cos(fr).T, (4, 1)))  # [128, T]
    sinT = np.ascontiguousarray(np.tile(np.sin(fr).T, (4, 1)))
    tri = np.triu(np.ones((128, 128), np.float32))

    hT = h.T                                             # [D, T]

    in_maps = []
    for c in range(NCORES):
        heads = list(range(HL * c, HL * (c + 1)))
        # phase_q chunk order: rope pair 0, rope pair 1, nope h0..h3;
        # each 128-col chunk packed independently (kernel slices per m)
        qchunks = []
        for pair in range(2):
            hh0, hh1 = heads[2 * pair], heads[2 * pair + 1]
            qchunks.append(np.concatenate([
                hh0 * QK + NOPE + np.arange(0, ROPE, 2),   # x1 h0
                hh1 * QK + NOPE + np.arange(0, ROPE, 2),   # x1 h1
                hh0 * QK + NOPE + np.arange(1, ROPE, 2),   # x2 h0
                hh1 * QK + NOPE + np.arange(1, ROPE, 2),   # x2 h1
            ]))
        for hh in heads:
            qchunks.append(np.arange(hh * QK, hh * QK + NOPE))
        wqb_packed = np.concatenate(
            [_pack_lhsT(wqb_eff[:, cols]) for cols in qchunks], axis=1)
        kcols = np.concatenate(
            [np.arange(hh * (NOPE + VH), hh * (NOPE + VH) + NOPE)
             for hh in heads])
        vcols = np.concatenate(
            [np.arange(hh * (NOPE + VH) + NOPE, (hh + 1) * (NOPE + VH))
             for hh in heads])
        hTc = hT[:, c * TC:(c + 1) * TC]
        in_maps.append({
            "hTp": _pack_lhsT(hTc),
            "wa": wa,
            "biask": bias,
            "wqb": wqb_packed,
            "wkvbk": _pack_lhsT(wkvb_eff[:, kcols]),
            "wkvbv": _pack_lhsT(wkvb_eff[:, vcols]),
            "wo": _pack_lhsT(Wo[:, c * DCOL:(c + 1) * DCOL]),
            "cosTb": cosT.astype(ml_dtypes.bfloat16),
            "sinTb": sinT.astype(ml_dtypes.bfloat16),
            "cosA": np.ascontiguousarray(cosT[:, c * TC:(c + 1) * TC]),
            "sinA": np.ascontiguousarray(sinT[:, c * TC:(c + 1) * TC]),
            "tri": tri,
            "onesin": np.ones((128, 128), np.float32),
        })
    return in_maps


def kernel(**inputs):
    nc = _get_nc()
    in_maps = _prep(inputs)
    res = bass_utils.run_bass_kernel_spmd(
        nc, in_maps, core_ids=list(range(NCORES)), trace=TRACE[0])
    LAST_RESULT[0] = res
    out = np.empty((T, D), np.float32)
    for c in range(NCORES):
        out[:, c * DCOL:(c + 1) * DCOL] = res.results[c]["outT"].T
    return out


# revision 24
# speedup vs baseline: 1.0028x; 1.0028x over previous
"""DeepseekV2 MLA attention on 8 Trainium2 NeuronCores.

Sharding: token-split A-projections (kv latents first -> early AllGather,
q latents second -> second AllGather) -> head-split (4 heads/core)
B-projections + causal attention -> per-query-block AllGather(attn out)
-> D-column-split output projection.

v2 layout notes vs baseline:
- All weights host-prepacked into the exact [128, k, cols] lhsT layout so
  every weight DMA is a contiguous stream.
- ag1 is split: kv latents (576 rows) gather while the q-chunk matmuls
  still run; q latents (1536 rows) gather while phase_b runs.
- qT stays resident in SBUF (no DRAM spill/reload); q rope is applied
  with 6 full-width DVE ops per (pair, qblock) writing qpe_sb directly.
- Attention denominators accumulate in bf16 on DVE (one ones-matmul per
  (qb, head) instead of one per (kk, head)); broadcast matmuls run bf16.
- Attention runs qb=3..0 so the last ag2 fires early and the output
  projection tail hides under remaining PE work.

Precision: bf16 matmul inputs with fp32 PSUM accumulation; rmsnorm
statistics in fp32/f32r; softmax denominators accumulate bf16 partials
into fp32 PSUM via a ones-matmul.
"""
import math

import numpy as np
import ml_dtypes

import concourse.bass as bass
import concourse.mybir as mybir
from concourse.tile import TileContext
from concourse import bass_utils

# ---------------------------------------------------------------------------
# Walrus workaround: this container's walrus accepts at most ONE sync-wait
# per TPB instruction, but Tile attaches several (tail Drain, LDWEIGHTS...).
# Split: keep the last wait, move the rest onto preceding same-engine NOPs.
# ---------------------------------------------------------------------------
import concourse.tile as _tile_mod

_orig_sched = _tile_mod.TileContext.schedule_and_allocate
_nopctr = [0]


def _split_multiwait(nc):
    for fn in nc.m.functions:
        for blk in fn.blocks:
            insts = blk.instructions
            if not any(
                i.sync_info and i.sync_info.on_wait and len(i.sync_info.on_wait) > 1
                for i in insts
            ):
                continue
            out = []
            for ins in insts:
                si = ins.sync_info
                if si and si.on_wait and len(si.on_wait) > 1:
                    waits = list(si.on_wait)
                    for w in waits[:-1]:
                        _nopctr[0] += 1
                        nop = mybir.InstNoOp(name=f"I-mws-{_nopctr[0]}", ins=[], outs=[])
                        nop.engine = ins.engine
                        nop.sync_info = mybir.SyncInfo(on_wait=[w], on_update=[])
                        out.append(nop)
                    ins.sync_info = mybir.SyncInfo(
                        on_wait=[waits[-1]], on_update=list(si.on_update or [])
                    )
                out.append(ins)
            blk.instructions = out


def _patched_sched(self, *a, **k):
    res = _orig_sched(self, *a, **k)
    _split_multiwait(self.nc)
    return res


if getattr(_tile_mod.TileContext.schedule_and_allocate, "__name__", "") != "_patched_sched":
    _tile_mod.TileContext.schedule_and_allocate = _patched_sched


# ---------------------------------------------------------------------------
T, D, H = 2048, 5120, 32
NOPE, ROPE, QK = 128, 64, 192
KVR, QR, VH = 512, 1536, 128
EPS, THETA = 1e-6, 10000.0
NCORES = 8
HL = H // NCORES          # 4 heads per core
TC = T // NCORES          # 256 tokens per core
LAT = KVR + ROPE          # 576 rows in the kv allgather
DCOL = D // NCORES        # 640 output columns per core
NKC = D // 128            # 40 contraction chunks for the A matmuls

F32 = mybir.dt.float32
F32R = mybir.dt.float32r
BF16 = mybir.dt.bfloat16
AF = mybir.ActivationFunctionType
MUL = mybir.AluOpType.mult
ADD = mybir.AluOpType.add
SUB = mybir.AluOpType.subtract

TRACE = [False]          # test.py sets TRACE[0]=True to profile
LAST_RESULT = [None]     # BassKernelResults stashed here for test.py

_cache = {}

# phase_a chunk table: (kind, mrows); chunk c covers wa cols
# [off(c), off(c)+40*mrows).  kv chunks first so ag1b can fire early.
_ACH = [("kv", 128)] * 4 + [("pe", 64)] + [("q", 128)] * 12
_AOFF = np.concatenate([[0], np.cumsum([NKC * m for _, m in _ACH])]).tolist()


def _phase_a_kv(nc, tc, io, consts_t, pools, ag1b_in):
    """Token-split kv-latent A projection, rmsnorm, k_pe rope -> ag1b_in."""
    a_ht, a_w, a_st, a_tmp, a_ps, a_ss = pools
    ones_cb, ones_r = consts_t["ones_cb"], consts_t["ones_r"]
    bias_sb = consts_t["bias_sb"]

    ht_sb = a_ht.tile([128, NKC * TC], BF16, name="ht_sb")
    htv = ht_sb[:].rearrange("p (k t) -> p k t", k=NKC)
    htp = io["hTp"][:].rearrange("p (k t) -> p k t", k=NKC)
    # DMA engines run well below peak while the device warms up, so the
    # first weight chunk and first ht chunk are finely interleaved: the
    # first matmuls become runnable after ~0.6MB instead of ~1.9MB.
    wt0 = a_w.tile([128, NKC * 128], BF16, name="a_w_0", tag="aw")
    nc.sync.dma_start(wt0[:, 0:5 * 128], io["wa"][:, 0:5 * 128])
    nc.sync.dma_start(htv[:, 0:4, :], htp[:, 0:4, :])
    nc.sync.dma_start(wt0[:, 5 * 128:10 * 128], io["wa"][:, 5 * 128:10 * 128])
    nc.sync.dma_start(htv[:, 4:8, :], htp[:, 4:8, :])
    for part in range(1, 4):
        lo, hi = part * (NKC // 4) * 128, (part + 1) * (NKC // 4) * 128
        nc.sync.dma_start(wt0[:, lo:hi], io["wa"][:, lo:hi])
    for g in range(1, 5):
        nc.sync.dma_start(htv[:, 8 * g:8 * (g + 1), :], htp[:, 8 * g:8 * (g + 1), :])
    stage = a_st.tile([128, 5 * TC], F32R, name="stage")
    ss_kv = a_ss.tile([1, TC], F32, name="ss_kv")
    consts_t["htv"] = htv
    consts_t["stage"] = stage

    for c in range(5):
        kind, mrows = _ACH[c]
        if c == 0:
            wt = wt0
        else:
            wt = a_w.tile([128, NKC * 128], BF16, name=f"a_w_{c}", tag="aw")
            nc.sync.dma_start(wt[:, :NKC * mrows],
                              io["wa"][:, _AOFF[c]:_AOFF[c + 1]])
        wtv = wt[:, :NKC * mrows].rearrange("p (k c) -> p k c", k=NKC)
        ps = a_ps.tile([128, TC], F32, name=f"a_ps_{c}", tag="aps")
        for k in range(NKC):
            nc.tensor.matmul(ps[:mrows, :], wtv[:, k, :], htv[:, k, :],
                             start=(k == 0), stop=(k == NKC - 1))
        if c == 0:
            # rope tables for the k_pe rope below; emitted here so they
            # trail the first weight chunk in the DMA queue
            for nm, srcn in (("cosa_sb", "cosA"), ("sina_sb", "sinA")):
                nc.sync.dma_start(consts_t[nm][:], io[srcn][:])
        st = stage[:, c * TC:(c + 1) * TC]
        if kind == "kv":
            nc.vector.tensor_scalar(st, ps[:], bias_sb[:, c:c + 1], None, op0=ADD)
            sq = a_tmp.tile([128, TC], BF16, name=f"sq_{c}", tag="sq")
            nc.scalar.activation(sq[:], st, AF.Square)
            nc.tensor.matmul(ss_kv[:], ones_cb, sq[:],
                             start=(c == 0), stop=(c == 3))
        else:
            nc.vector.tensor_scalar(st[:64, :], ps[:64, :],
                                    bias_sb[:64, 4:5], None, op0=ADD)

    # kv rms scale: 1/sqrt(mean(ss) + eps) broadcast to 128 partitions
    bc_kv = _rms_scale(nc, consts_t, a_tmp, a_ps, ss_kv, KVR, "kv")
    for c in range(4):
        st = stage[:, c * TC:(c + 1) * TC]
        sc = a_tmp.tile([128, TC], BF16, name=f"sc_kv_{c}", tag="sc")
        nc.vector.tensor_tensor(sc[:], st, bc_kv[:], op=MUL)
        nc.sync.dma_start(ag1b_in[c * 128:(c + 1) * 128, :], sc[:])

    # k_pe rope (no norm) -> rows 512:576
    cosa_sb, sina_sb = consts_t["cosa_sb"], consts_t["sina_sb"]
    st = stage[:, 4 * TC:5 * TC]
    rp = a_tmp.tile([64, TC], BF16, name="rp_kpe")
    t1 = a_tmp.tile([32, TC], F32R, name="rt1", tag="rt1")
    t2 = a_tmp.tile([32, TC], F32R, name="rt2", tag="rt2")
    x1, x2 = st[0:32, :], st[32:64, :]
    nc.vector.tensor_tensor(t1[:], x1, cosa_sb[0:32, :], op=MUL)
    nc.vector.tensor_tensor(t2[:], x2, sina_sb[32:64, :], op=MUL)
    nc.vector.tensor_tensor(rp[0:32, :], t1[:], t2[:], op=SUB)
    nc.vector.tensor_tensor(t1[:], x1, sina_sb[0:32, :], op=MUL)
    nc.vector.tensor_tensor(t2[:], x2, cosa_sb[32:64, :], op=MUL)
    nc.vector.tensor_tensor(rp[32:64, :], t1[:], t2[:], op=ADD)
    nc.sync.dma_start(ag1b_in[512:576, :], rp[:])


def _phase_a_q(nc, tc, io, consts_t, pools, ag1a1_in, ag1a2_in, fire1):
    """Token-split q-latent A projection, staged UNNORMALIZED.

    The per-token rms scale rs ships as row 768 of ag1a2; phase_q applies
    it after its matmuls (the norm is linear in the latent).  fire1() is
    called once the first six chunks are staged, so the first half
    gathers while the rest computes.
    """
    a_ht, a_w, a_st, a_tmp, a_ps, a_ss = pools
    ones_cb = consts_t["ones_cb"]
    htv = consts_t["htv"]
    ss_q = a_ss.tile([1, TC], F32, name="ss_q")

    for c in range(5, 17):
        wt = a_w.tile([128, NKC * 128], BF16, name=f"a_w_{c}", tag="aw")
        nc.sync.dma_start(wt[:], io["wa"][:, _AOFF[c]:_AOFF[c + 1]])
        wtv = wt[:].rearrange("p (k c) -> p k c", k=NKC)
        ps = a_ps.tile([128, TC], F32, name=f"a_ps_{c}", tag="aps")
        for k in range(NKC):
            nc.tensor.matmul(ps[:], wtv[:, k, :], htv[:, k, :],
                             start=(k == 0), stop=(k == NKC - 1))
        sq = a_tmp.tile([128, TC], BF16, name=f"sq_{c}", tag="sq")
        nc.scalar.activation(sq[:], ps[:], AF.Square)
        nc.tensor.matmul(ss_q[:], ones_cb, sq[:],
                         start=(c == 5), stop=(c == 16))
        sc = a_tmp.tile([128, TC], BF16, name=f"sc_q_{c}", tag="sc")
        nc.vector.tensor_copy(sc[:], ps[:])
        if c < 11:
            nc.sync.dma_start(ag1a1_in[(c - 5) * 128:(c - 4) * 128, :], sc[:])
            if c == 10:
                fire1()
        else:
            nc.sync.dma_start(ag1a2_in[(c - 11) * 128:(c - 10) * 128, :], sc[:])

    # rs = 1/sqrt(mean(ss)+eps), shipped bf16 as row 768 of ag1a2
    ms = a_tmp.tile([1, TC], F32R, name="ms_q", tag="ms")
    nc.vector.tensor_scalar(ms[:], ss_q[:], 1.0 / QR, EPS, op0=MUL, op1=ADD)
    sq2 = a_tmp.tile([1, TC], F32R, name="sqr_q", tag="sqr")
    nc.scalar.activation(sq2[:], ms[:], AF.Sqrt)
    rs = a_tmp.tile([1, TC], F32R, name="rs_q", tag="rs")
    with nc.allow_low_precision(reason="f32r holds full fp32 bits"):
        nc.vector.reciprocal(rs[:], sq2[:])
    rsb = a_tmp.tile([1, TC], BF16, name="rsb_q", tag="rsb")
    nc.vector.tensor_copy(rsb[:], rs[:])
    nc.sync.dma_start(ag1a2_in[768:769, :], rsb[:])


def _rms_scale(nc, consts_t, a_tmp, a_ps, ss, nfeat, key):
    """1/sqrt(mean(ss)+eps) broadcast to [128, TC] f32r."""
    ms = a_tmp.tile([1, TC], F32R, name=f"ms_{key}", tag="ms")
    nc.vector.tensor_scalar(ms[:], ss[:], 1.0 / nfeat, EPS, op0=MUL, op1=ADD)
    sq2 = a_tmp.tile([1, TC], F32R, name=f"sqr_{key}", tag="sqr")
    nc.scalar.activation(sq2[:], ms[:], AF.Sqrt)
    rs = a_tmp.tile([1, TC], F32R, name=f"rs_{key}", tag="rs")
    with nc.allow_low_precision(reason="f32r holds full fp32 bits"):
        nc.vector.reciprocal(rs[:], sq2[:])
    bps = a_ps.tile([128, TC], F32, name=f"bps_{key}", tag="bps")
    nc.tensor.matmul(bps[:], consts_t["ones_r"][:1, :], rs[:],
                     start=True, stop=True)
    bc = a_tmp.tile([128, TC], F32R, name=f"bc_{key}", tag=f"bc{key}")
    nc.vector.tensor_copy(bc[:], bps[:])
    return bc


def _phase_b(nc, tc, io, ag1bv, ktv, vv, kpe_sb):
    """Head-split k_nope^T and v projections from the gathered kv latents."""
    with (
        tc.tile_pool(name="b_kva", bufs=1) as b_kva,
        tc.tile_pool(name="b_w", bufs=2) as b_w,
        tc.tile_pool(name="b_ps", bufs=2, space="PSUM") as b_ps,
    ):
        kva_sb = b_kva.tile([128, 4 * T], BF16, name="kva_sb")
        kvav = kva_sb[:].rearrange("p (k t) -> p k t", k=4)
        for k in range(4):
            nc.gpsimd.dma_start(
                kvav[:, k, :].rearrange("p (r t) -> p r t", r=NCORES),
                ag1bv[k * 128:(k + 1) * 128])
        nc.gpsimd.dma_start(
            kpe_sb[:].rearrange("p (r t) -> p r t", r=NCORES),
            ag1bv[512:576])

        wk_sb = b_w.tile([128, 4 * 512], BF16, name="wk_sb", tag="wkw")
        wkv_ = wk_sb[:].rearrange("p (k c) -> p k c", k=4)
        nc.sync.dma_start(wk_sb[:], io["wkvbk"][:])
        for j in range(HL):
            for qb in range(4):
                ps = b_ps.tile([128, 512], F32, name=f"psk_{j}_{qb}", tag="psk")
                for k in range(4):
                    nc.tensor.matmul(ps[:], wkv_[:, k, j * 128:(j + 1) * 128],
                                     kvav[:, k, qb * 512:(qb + 1) * 512],
                                     start=(k == 0), stop=(k == 3))
                nc.vector.tensor_copy(ktv[:, j, qb * 512:(qb + 1) * 512], ps[:])

        wv_sb = b_w.tile([128, 4 * 512], BF16, name="wv_sb", tag="wvw")
        wvv = wv_sb[:].rearrange("p (k c) -> p k c", k=4)
        nc.sync.dma_start(wv_sb[:], io["wkvbv"][:])
        for mt in range(16):
            ps = b_ps.tile([128, 512], F32, name=f"psv_{mt}", tag="psv")
            for k in range(4):
                nc.tensor.matmul(ps[:], kvav[:, k, mt * 128:(mt + 1) * 128],
                                 wvv[:, k, :], start=(k == 0), stop=(k == 3))
            nc.vector.tensor_copy(vv[:, mt, :], ps[:])


def _phase_q(nc, tc, io, consts_t, ag1a1v, ag1a2v, qtv, qpev):
    """Head-split q^T projection from unnormalized latents; rope pairs
    first; the per-token rms scale rs is broadcast and applied here."""
    cos_sb, sin_sb = consts_t["cos_sb"], consts_t["sin_sb"]
    ones_rb = consts_t["ones_rb"]
    with (
        tc.tile_pool(name="c_qa", bufs=1) as c_qa,
        tc.tile_pool(name="c_w", bufs=2) as c_w,
        tc.tile_pool(name="c_tmp", bufs=3) as c_tmp,
        tc.tile_pool(name="c_rs", bufs=1) as c_rs,
        tc.tile_pool(name="c_ps", bufs=3, space="PSUM") as c_ps,
        tc.tile_pool(name="c_bps", bufs=1, space="PSUM") as c_bps,
    ):
        qa_sb = c_qa.tile([128, 12 * T], BF16, name="qa_sb")
        qav = qa_sb[:].rearrange("p (k t) -> p k t", k=12)
        for k in range(12):
            srcv = (ag1a1v[k * 128:(k + 1) * 128] if k < 6 else
                    ag1a2v[(k - 6) * 128:(k - 5) * 128])
            nc.gpsimd.dma_start(
                qav[:, k, :].rearrange("p (r t) -> p r t", r=NCORES), srcv)
        rs_sb = c_rs.tile([1, T], BF16, name="rs_sb")
        nc.gpsimd.dma_start(
            rs_sb[:].rearrange("p (r t) -> p r t", r=NCORES),
            ag1a2v[768:769])
        rsb = c_rs.tile([128, T], BF16, name="rsb")

        for m in range(6):
            wt = c_w.tile([128, 12 * 128], BF16, name=f"cw_{m}", tag="cw")
            nc.sync.dma_start(wt[:], io["wqb"][:, m * 1536:(m + 1) * 1536])
            wtv = wt[:].rearrange("p (k c) -> p k c", k=12)
            for pair in range(2):
                pss = [c_ps.tile([128, 512], F32, name=f"psq_{m}_{qb}",
                                 tag=f"psq{qb % 2}")
                       for qb in (2 * pair, 2 * pair + 1)]
                for k in range(12):
                    for s in range(2):
                        qb = 2 * pair + s
                        nc.tensor.matmul(pss[s][:], wtv[:, k, :],
                                         qav[:, k, qb * 512:(qb + 1) * 512],
                                         start=(k == 0), stop=(k == 11))
                if m == 0 and pair == 0:
                    # rs broadcast to 128 partitions, once; then folded
                    # straight into the rope tables so each rope block
                    # saves two full-width DVE muls
                    for b in range(4):
                        bp = c_bps.tile([128, 512], F32, name=f"rsbp_{b}",
                                        tag="rsbp")
                        nc.tensor.matmul(bp[:], ones_rb[:1, :],
                                         rs_sb[:, b * 512:(b + 1) * 512],
                                         start=True, stop=True)
                        nc.vector.tensor_copy(
                            rsb[:, b * 512:(b + 1) * 512], bp[:])
                    nc.vector.tensor_tensor(cos_sb[:], cos_sb[:], rsb[:],
                                            op=MUL)
                    nc.vector.tensor_tensor(sin_sb[:], sin_sb[:], rsb[:],
                                            op=MUL)
                for s in range(2):
                    qb = 2 * pair + s
                    cols = slice(qb * 512, (qb + 1) * 512)
                    if m < 2:
                        # rope pair m: heads (2m, 2m+1); pss rows are
                        # [x1h0, x1h1, x2h0, x2h1].  A,B carry the rs
                        # scale; Bs halves are swapped so every combine
                        # reads both inputs from the same base partition.
                        A = c_tmp.tile([128, 512], F32R, name=f"ra_{m}_{qb}",
                                       tag="ra")
                        B = c_tmp.tile([128, 512], F32R, name=f"rb_{m}_{qb}",
                                       tag="rb")
                        nc.vector.tensor_tensor(A[:], pss[s][:],
                                                cos_sb[:, cols], op=MUL)
                        nc.vector.tensor_tensor(B[0:64, :], pss[s][64:128, :],
                                                sin_sb[64:128, cols], op=MUL)
                        nc.vector.tensor_tensor(B[64:128, :], pss[s][0:64, :],
                                                sin_sb[0:64, cols], op=MUL)
                        for hh in range(2):
                            j = 2 * m + hh
                            r0 = 32 * hh
                            nc.vector.tensor_tensor(
                                qpev[0:32, j, cols], A[r0:r0 + 32, :],
                                B[r0:r0 + 32, :], op=SUB)
                            nc.vector.tensor_tensor(
                                qpev[32:64, j, cols], A[64 + r0:64 + r0 + 32, :],
                                B[64 + r0:64 + r0 + 32, :], op=ADD)
                    else:
                        nc.vector.tensor_tensor(qtv[:, m - 2, cols],
                                                pss[s][:], rsb[:, cols],
                                                op=MUL)


def _phase_attn_out(nc, tc, io, qtv, qpev, ag2_ins, ag2_outs, ktv, vv,
                    kpe_sb, wov, consts_t):
    """Causal attention (qb descending, software-pipelined ots) with the
    output-projection token blocks interleaved into the PE issue order:
    qb3, qb2, OUT3, qb1, OUT2, qb0, OUT1, OUT0."""
    ones_cb, ones_rb, tri_sb = (consts_t["ones_cb"], consts_t["ones_rb"],
                                consts_t["tri_b"])
    with (
        tc.tile_pool(name="t_p", bufs=6) as t_p,
        tc.tile_pool(name="t_den", bufs=2) as t_den,
        tc.tile_pool(name="t_o", bufs=2) as t_o,
        tc.tile_pool(name="t_ps", bufs=3, space="PSUM") as t_ps,
        tc.tile_pool(name="t_db", bufs=1, space="PSUM") as t_db,
        tc.tile_pool(name="t_acc", bufs=1, space="PSUM") as t_acc,
        tc.tile_pool(name="o_a", bufs=3) as o_a,
        tc.tile_pool(name="o_st", bufs=2) as o_st,
        tc.tile_pool(name="o_ps", bufs=2, space="PSUM") as o_ps,
    ):
        def attn_qb(qb, fillers=()):
            fillers = list(fillers)
            for jp in range(HL // 2):
                js = (2 * jp, 2 * jp + 1)
                dacs, ots, prev = {}, {}, {}
                for s, j in enumerate(js):
                    dacs[j] = t_den.tile([128, 512], BF16, name=f"dac_{qb}_{j}",
                                         tag=f"dac{s}")
                    ots[j] = t_acc.tile([128, 512], F32, name=f"ot_{qb}_{j}",
                                        tag=f"ot{s}")
                kmax = 4 * qb + 4
                for kk in range(kmax):
                    o = kk - 4 * qb
                    c0 = max(0, o) * 128
                    # both heads' nope matmuls first, then both rope
                    # matmuls: consecutive PE ops hit alternating PSUM
                    # banks, so drain overlaps the next op's fill
                    sTs = {}
                    for s, j in enumerate(js):
                        sT = t_ps.tile([128, 512], F32,
                                       name=f"sT_{qb}_{j}_{kk}", tag="sT")
                        nc.tensor.matmul(sT[:, c0:512],
                                         ktv[:, j, kk * 128:(kk + 1) * 128],
                                         qtv[:, j, qb * 512 + c0:(qb + 1) * 512],
                                         start=True, stop=False)
                        sTs[j] = sT
                    pts = {}
                    for s, j in enumerate(js):
                        sT = sTs[j]
                        nc.tensor.matmul(sT[:, c0:512],
                                         kpe_sb[:, kk * 128:(kk + 1) * 128],
                                         qpev[:, j, qb * 512 + c0:(qb + 1) * 512],
                                         start=False, stop=True)
                        pT = t_p.tile([128, 512], BF16,
                                      name=f"pT_{qb}_{j}_{kk}", tag="pT")
                        nc.scalar.activation(pT[:, c0:512], sT[:, c0:512],
                                             AF.Exp)
                        if o >= 0:
                            nc.vector.tensor_tensor(pT[:, c0:c0 + 128],
                                                    pT[:, c0:c0 + 128],
                                                    tri_sb[:], op=MUL)
                        pts[j] = pT
                    # deferred PV matmuls from the previous kk so the exp
                    # (Scalar) of this kk hides under PE work
                    if kk > 0:
                        for j in js:
                            pc0, pT0 = prev[j]
                            nc.tensor.matmul(ots[j][:, pc0:512],
                                             vv[:, kk - 1, j * 128:(j + 1) * 128],
                                             pT0[:, pc0:512],
                                             start=(kk == 1), stop=False)
                    for j in js:
                        pT = pts[j]
                        if kk == 0:
                            nc.vector.tensor_copy(dacs[j][:], pT[:])
                        else:
                            nc.vector.tensor_tensor(dacs[j][:, c0:512],
                                                    dacs[j][:, c0:512],
                                                    pT[:, c0:512], op=ADD)
                        prev[j] = (c0, pT)
                for j in js:
                    pc0, pT0 = prev[j]
                    nc.tensor.matmul(ots[j][:, pc0:512],
                                     vv[:, kmax - 1, j * 128:(j + 1) * 128],
                                     pT0[:, pc0:512],
                                     start=(kmax == 1), stop=True)
                if fillers:
                    f = fillers.pop(0)
                    if f:
                        f()
                for s, j in enumerate(js):
                    denp = t_db.tile([1, 512], F32, name=f"dp_{qb}_{j}",
                                     tag="db")
                    nc.tensor.matmul(denp[:], ones_cb, dacs[j][:],
                                     start=True, stop=True)
                    rden = t_o.tile([1, 512], F32R, name=f"rden_{qb}_{j}",
                                    tag=f"rden{s}")
                    with nc.allow_low_precision(reason="f32r = fp32 bits"):
                        nc.vector.reciprocal(rden[:], denp[:])
                    rdb = t_o.tile([1, 512], BF16, name=f"rdb_{qb}_{j}",
                                   tag=f"rdb{s}")
                    nc.vector.tensor_copy(rdb[:], rden[:])
                    bcp = t_db.tile([128, 512], F32, name=f"bcp_{qb}_{j}",
                                    tag="db")
                    nc.tensor.matmul(bcp[:], ones_rb[:1, :], rdb[:],
                                     start=True, stop=True)
                    bcs = t_o.tile([128, 512], F32R, name=f"bcs_{qb}_{j}",
                                   tag=f"bcs{s}")
                    nc.vector.tensor_copy(bcs[:], bcp[:])
                    obf = t_o.tile([128, 512], BF16, name=f"obf_{qb}_{j}",
                                   tag=f"obf{s}")
                    nc.vector.tensor_tensor(obf[:], ots[j][:], bcs[:], op=MUL)
                    nc.sync.dma_start(
                        ag2_ins[qb][j * 128:(j + 1) * 128, :], obf[:])
            nc.gpsimd.collective_compute(
                "AllGather", mybir.AluOpType.bypass,
                ins=[ag2_ins[qb][:]], outs=[ag2_outs[qb][:]],
                replica_groups=[list(range(NCORES))],
            )

        def out_prep(tq):
            oavs = []
            for h in range(2):
                oa = o_a.tile([128, 16 * 512], BF16, name=f"oa_{tq}_{h}",
                              tag="oa")
                oav = oa[:].rearrange("p (k t) -> p k t", k=16)
                nc.gpsimd.dma_start(
                    oav, ag2_outs[tq][:].rearrange("(k p) t -> p k t", p=128)
                    [:, 16 * h:16 * (h + 1), :])
                oavs.append(oav)
            return oavs

        def out_d(tq, oavs, d):
            ps = o_ps.tile([128, 512], F32, name=f"ops_{tq}_{d}", tag="ops")
            for k in range(32):
                nc.tensor.matmul(ps[:], wov[:, k, d * 128:(d + 1) * 128],
                                 oavs[k // 16][:, k % 16, :],
                                 start=(k == 0), stop=(k == 31))
            st = o_st.tile([128, 512], F32, name=f"ost_{tq}_{d}", tag="ost")
            nc.vector.tensor_copy(st[:], ps[:])
            nc.sync.dma_start(
                io["outT"][d * 128:(d + 1) * 128,
                           tq * 512:(tq + 1) * 512], st[:])

        attn_qb(3)
        p3 = out_prep(3)
        attn_qb(2, [None, lambda: out_d(3, p3, 0)])
        attn_qb(1, [lambda: out_d(3, p3, 1), lambda: out_d(3, p3, 2)])
        p2 = out_prep(2)
        attn_qb(0, [lambda: out_d(3, p3, 3), lambda: out_d(3, p3, 4)])
        for d in range(5):
            out_d(2, p2, d)
        p1 = out_prep(1)
        for d in range(5):
            out_d(1, p1, d)
        p0 = out_prep(0)
        for d in range(5):
            out_d(0, p0, d)


def _build():
    nc = bass.Bass("TRN2", target_bir_lowering=False, debug=False,
                   num_devices=NCORES)
    io = {
        "hTp": nc.dram_tensor("hTp", [128, NKC * TC], BF16, kind="ExternalInput"),
        "wa": nc.dram_tensor("wa", [128, _AOFF[-1]], BF16, kind="ExternalInput"),
        "biask": nc.dram_tensor("biask", [128, 5], F32, kind="ExternalInput"),
        "wqb": nc.dram_tensor("wqb", [128, 6 * 1536], BF16, kind="ExternalInput"),
        "wkvbk": nc.dram_tensor("wkvbk", [128, 4 * 512], BF16,
                                kind="ExternalInput"),
        "wkvbv": nc.dram_tensor("wkvbv", [128, 4 * 512], BF16,
                                kind="ExternalInput"),
        "wo": nc.dram_tensor("wo", [128, 32 * DCOL], BF16, kind="ExternalInput"),
        "cosTb": nc.dram_tensor("cosTb", [128, T], BF16, kind="ExternalInput"),
        "sinTb": nc.dram_tensor("sinTb", [128, T], BF16, kind="ExternalInput"),
        "cosA": nc.dram_tensor("cosA", [128, TC], F32R, kind="ExternalInput"),
        "sinA": nc.dram_tensor("sinA", [128, TC], F32R, kind="ExternalInput"),
        "tri": nc.dram_tensor("tri", [128, 128], F32R, kind="ExternalInput"),
        "onesin": nc.dram_tensor("onesin", [128, 128], F32R, kind="ExternalInput"),
        "outT": nc.dram_tensor("outT", [DCOL, T], F32, kind="ExternalOutput"),
    }

    with TileContext(nc) as tc:
        with (
            tc.tile_pool(name="dram", bufs=1, space="DRAM") as dram,
            tc.tile_pool(name="consts", bufs=1) as consts,
            tc.tile_pool(name="persist", bufs=1) as persist,
        ):
            dum_in = dram.tile([1, 16], BF16, name="dum_in")
            dum_out = dram.tile([NCORES, 16], BF16, addr_space="Shared",
                                name="dum_out")
            ag1a1_in = dram.tile([768, TC], BF16, name="ag1a1_in")
            ag1a1_out = dram.tile([NCORES * 768, TC], BF16, addr_space="Shared",
                                  name="ag1a1_out")
            ag1a2_in = dram.tile([769, TC], BF16, name="ag1a2_in")
            ag1a2_out = dram.tile([NCORES * 769, TC], BF16, addr_space="Shared",
                                  name="ag1a2_out")
            ag1b_in = dram.tile([LAT, TC], BF16, name="ag1b_in")
            ag1b_out = dram.tile([NCORES * LAT, TC], BF16, addr_space="Shared",
                                 name="ag1b_out")
            ag2_ins = [dram.tile([HL * VH, 512], BF16, name=f"ag2_in_{qb}")
                       for qb in range(4)]
            ag2_outs = [dram.tile([H * VH, 512], BF16, addr_space="Shared",
                                  name=f"ag2_out_{qb}") for qb in range(4)]

            def fire_ag(name, tin, tout):
                with nc.named_scope(name):
                    nc.gpsimd.collective_compute(
                        "AllGather", mybir.AluOpType.bypass,
                        ins=[tin[:]], outs=[tout[:]],
                        replica_groups=[list(range(NCORES))],
                    )

            # absorb cross-core launch skew before real collectives
            zb = consts.tile([1, 16], BF16, name="zb")
            nc.vector.memset(zb[:], 0)
            nc.sync.dma_start(dum_in[:], zb[:])
            fire_ag("dummy", dum_in, dum_out)

            consts_t = {}
            ones_sb = consts.tile([128, 128], F32R, name="ones_sb")
            nc.sync.dma_start(ones_sb[:], io["onesin"][:])
            consts_t["ones_r"] = ones_sb
            ones_b = consts.tile([128, 128], BF16, name="ones_b")
            nc.vector.tensor_copy(ones_b[:], ones_sb[:])
            consts_t["ones_cb"] = ones_b[:, 0:1]
            consts_t["ones_rb"] = ones_b
            consts_t["bias_sb"] = consts.tile([128, 5], F32, name="bias_sb")
            nc.sync.dma_start(consts_t["bias_sb"][:], io["biask"][:])
            for nm, shp in (("tri_sb", [128, 128]),
                            ("cosa_sb", [128, TC]),
                            ("sina_sb", [128, TC])):
                consts_t[nm] = consts.tile(shp, F32R, name=nm)
            trib = consts.tile([128, 128], BF16, name="trib")
            consts_t["tri_b"] = trib

            # persistent SBUF tensors
            kt_sb = persist.tile([128, HL * T], BF16, name="kt_sb")
            ktv = kt_sb[:].rearrange("p (j t) -> p j t", j=HL)
            v_sb = persist.tile([128, 16 * 512], BF16, name="v_sb")
            vv = v_sb[:].rearrange("p (mt c) -> p mt c", mt=16)
            kpe_sb = persist.tile([64, T], BF16, name="kpe_sb")
            qt_sb = persist.tile([128, 4 * T], BF16, name="qt_sb")
            qtv = qt_sb[:].rearrange("p (j t) -> p j t", j=4)
            qpe_sb = persist.tile([64, HL * T], BF16, name="qpe_sb")
            qpev = qpe_sb[:].rearrange("p (j t) -> p j t", j=HL)
            wo_sb = persist.tile([128, 32 * DCOL], BF16, name="wo_sb")
            wov = wo_sb[:].rearrange("p (k c) -> p k c", k=32)

            with (
                tc.tile_pool(name="a_ht", bufs=1) as a_ht,
                tc.tile_pool(name="a_w", bufs=3) as a_w,
                tc.tile_pool(name="a_st", bufs=1) as a_st,
                tc.tile_pool(name="a_tmp", bufs=2) as a_tmp,
                tc.tile_pool(name="a_ps", bufs=2, space="PSUM") as a_ps,
                tc.tile_pool(name="a_ss", bufs=1, space="PSUM") as a_ss,
            ):
                pools = (a_ht, a_w, a_st, a_tmp, a_ps, a_ss)
                with nc.named_scope("phase_a"):
                    _phase_a_kv(nc, tc, io, consts_t, pools, ag1b_in)
                fire_ag("ag1b", ag1b_in, ag1b_out)
                with nc.named_scope("phase_a2"):
                    _phase_a_q(nc, tc, io, consts_t, pools, ag1a1_in,
                               ag1a2_in,
                               lambda: fire_ag("ag1a1", ag1a1_in, ag1a1_out))
                fire_ag("ag1a2", ag1a2_in, ag1a2_out)
            ag1a1v = ag1a1_out[:].rearrange("(r a) t -> a r t", a=768)
            ag1a2v = ag1a2_out[:].rearrange("(r a) t -> a r t", a=769)
            ag1bv = ag1b_out[:].rearrange("(r a) t -> a r t", a=LAT)

            nc.gpsimd.dma_start(wo_sb[:], io["wo"][:])
            with nc.named_scope("phase_b"):
                _phase_b(nc, tc, io, ag1bv, ktv, vv, kpe_sb)
            # late consts: rope tables for phase_q, mask for attn
            for nm, srcn in (("cos_sb", "cosTb"), ("sin_sb", "sinTb")):
                consts_t[nm] = consts.tile([128, T], BF16, name=nm)
                nc.sync.dma_start(consts_t[nm][:], io[srcn][:])
            nc.sync.dma_start(consts_t["tri_sb"][:], io["tri"][:])
            nc.vector.tensor_copy(trib[:], consts_t["tri_sb"][:])
            with nc.named_scope("phase_q"):
                _phase_q(nc, tc, io, consts_t, ag1a1v, ag1a2v, qtv, qpev)
            with nc.named_scope("phase_attn"):
                _phase_attn_out(nc, tc, io, qtv, qpev, ag2_ins, ag2_outs,
                                ktv, vv, kpe_sb, wov, consts_t)
    return nc


def _get_nc():
    if "nc" not in _cache:
        _cache["nc"] = _build()
    return _cache["nc"]


def _pack_lhsT(w, ncols=None):
    """[D, C] weight -> [128, (D/128)*C] bf16, k-major per-partition cols."""
    d, c = w.shape
    arr = w.reshape(d // 128, 128, c).transpose(1, 0, 2).reshape(128, -1)
    return np.ascontiguousarray(arr).astype(ml_dtypes.bfloat16)


def _prep(inputs):
    h = np.asarray(inputs["h"], np.float32)
    pos = np.asarray(inputs["position_ids"], np.int32)
    Wq_a = np.asarray(inputs["Wq_a"], np.float32)
    gq = np.asarray(inputs["gq"], np.float32)
    Wq_b = np.asarray(inputs["Wq_b"], np.float32)
    Wkv_a = np.asarray(inputs["Wkv_a"], np.float32)
    bkv_a = np.asarray(inputs["bkv_a"], np.float32)
    gkv = np.asarray(inputs["gkv"], np.float32)
    Wkv_b = np.asarray(inputs["Wkv_b"], np.float32)
    Wo = np.asarray(inputs["Wo"], np.float32)

    dperm = np.concatenate([np.arange(0, ROPE, 2), np.arange(1, ROPE, 2)])
    scale = np.float32(1.0 / math.sqrt(QK))

    # phase_a weights: kv chunks, k_pe chunk, q chunks (packed lhsT layout)
    wa_chunks = []
    for c in range(4):
        wa_chunks.append(_pack_lhsT(Wkv_a[:, c * 128:(c + 1) * 128]))
    wa_chunks.append(_pack_lhsT(Wkv_a[:, KVR + dperm]))
    for c in range(12):
        wa_chunks.append(_pack_lhsT(Wq_a[:, c * 128:(c + 1) * 128]))
    wa = np.concatenate(wa_chunks, axis=1)

    bias = np.zeros((128, 5), np.float32)
    bias[:, :4] = bkv_a[:KVR].reshape(4, 128).T
    bias[:64, 4] = bkv_a[KVR + dperm]

    wqb_eff = (Wq_b * gq[:, None]) * scale              # [QR, H*QK]
    wkvb_eff = Wkv_b * gkv[:, None]                     # [KVR, H*(NOPE+VH)]

    inv = THETA ** (-np.arange(0, ROPE, 2, dtype=np.float32) / ROPE)
    fr = pos.astype(np.float32)[:, None] * inv[None, :]  # [T, 32]
    cosT = np.ascontiguousarray(np.tile(np.cos(fr).T, (4, 1)))  # [128, T]
    sinT = np.ascontiguousarray(np.tile(np.sin(fr).T, (4, 1)))
    tri = np.triu(np.ones((128, 128), np.float32))

    hT = h.T                                             # [D, T]

    in_maps = []
    for c in range(NCORES):
        heads = list(range(HL * c, HL * (c + 1)))
        # phase_q chunk order: rope pair 0, rope pair 1, nope h0..h3;
        # each 128-col chunk packed independently (kernel slices per m)
        qchunks = []
        for pair in range(2):
            hh0, hh1 = heads[2 * pair], heads[2 * pair + 1]
            qchunks.append(np.concatenate([
                hh0 * QK + NOPE + np.arange(0, ROPE, 2),   # x1 h0
                hh1 * QK + NOPE + np.arange(0, ROPE, 2),   # x1 h1
                hh0 * QK + NOPE + np.arange(1, ROPE, 2),   # x2 h0
                hh1 * QK + NOPE + np.arange(1, ROPE, 2),   # x2 h1
            ]))
        for hh in heads:
            qchunks.append(np.arange(hh * QK, hh * QK + NOPE))
        wqb_packed = np.concatenate(
            [_pack_lhsT(wqb_eff[:, cols]) for cols in qchunks], axis=1)
        kcols = np.concatenate(
            [np.arange(hh * (NOPE + VH), hh * (NOPE + VH) + NOPE)
             for hh in heads])
        vcols = np.concatenate(
            [np.arange(hh * (NOPE + VH) + NOPE, (hh + 1) * (NOPE + VH))
             for hh in heads])
        hTc = hT[:, c * TC:(c + 1) * TC]
        in_maps.append({
            "hTp": _pack_lhsT(hTc),
            "wa": wa,
            "biask": bias,
            "wqb": wqb_packed,
            "wkvbk": _pack_lhsT(wkvb_eff[:, kcols]),
            "wkvbv": _pack_lhsT(wkvb_eff[:, vcols]),
            "wo": _pack_lhsT(Wo[:, c * DCOL:(c + 1) * DCOL]),
            "cosTb": cosT.astype(ml_dtypes.bfloat16),
            "sinTb": sinT.astype(ml_dtypes.bfloat16),
            "cosA": np.ascontiguousarray(cosT[:, c * TC:(c + 1) * TC]),
            "sinA": np.ascontiguousarray(sinT[:, c * TC:(c + 1) * TC]),
            "tri": tri,
            "onesin": np.ones((128, 128), np.float32),
        })
    return in_maps


def kernel(**inputs):
    nc = _get_nc()
    in_maps = _prep(inputs)
    res = bass_utils.run_bass_kernel_spmd(
        nc, in_maps, core_ids=list(range(NCORES)), trace=TRACE[0])
    LAST_RESULT[0] = res
    out = np.empty((T, D), np.float32)
    for c in range(NCORES):
        out[:, c * DCOL:(c + 1) * DCOL] = res.results[c]["outT"].T
    return out
